# revision 6
# baseline (speedup 1.0000x reference)
"""Trainium2 Bass kernel for sliding-window GQA attention block.

Problem: B=1, S=2048, HID=2048, NH=16 q-heads, NKV=4 kv-heads, HD=128,
WINDOW=512, causal; rotary embedding on q/k; projections wq/wk/wv/wo.

Sharding (8 cores): tensor-parallel over the 4 KV-head groups (4 q-heads
per group) x sequence-parallel over 2 halves of 1024 queries. Each core
computes its group's q/k/v projections for its sequence span (+512-key
halo), banded sliding-window attention, and a partial output projection
(emitted transposed, [od, q]). Host sums the 4 group-partials per half.

All tensors are bf16 on the wire and as matmul operands (PSUM accum is
f32): halves DMA vs fp32, runs 1 cycle/row on the PE at any N, and gives
2x DVE throughput. x is loaded once and stays resident in SBUF for both
the k/v and q projections. Phases are pipelined per-head so RoPE (DVE)
and exp (Act) hide under PE matmuls. Scores windows are exact (no f32r
N>=256 padding needed with bf16). Masking is geometric: two 0/1 boundary
tiles applied to exp(scores), plus zeroed rows in the all-ones
denominator matmul for the r=0 halo padding.
"""
import sys
import os

sys.path.insert(0, "/opt/trn_rl_repo")

import numpy as np
import ml_dtypes

import concourse.bass as bass
import concourse.mybir as mybir
from concourse import bacc
import concourse.tile as tile
from concourse.bass_utils import run_bass_kernel_spmd

BF16 = mybir.dt.bfloat16
F32 = mybir.dt.float32
NPBF16 = ml_dtypes.bfloat16

S, HID, NH, NKV, HD, WINDOW = 2048, 2048, 16, 4, 128, 512
NCORES = 8
SQ = 1024          # queries per core
SK = 1536          # keys per core (incl. 512 halo)
HT = HID // 128    # 16 hid tiles
NHC = NH // NKV    # 4 q-heads per core
TJ = SK // 128     # 12 key tiles
EXP = mybir.ActivationFunctionType.Exp


def _win(tj):
    """Query window [w0, w1) of key tile tj in core-local coordinates."""
    return max(0, 128 * tj - 512), min(SQ, 128 * tj + 128)


_P_OFF = []
_off = 0
for _tj in range(TJ):
    _w0, _w1 = _win(_tj)
    _P_OFF.append(_off)
    _off += _w1 - _w0
P_TOTAL = _off  # 5120


def build_nc():
    nc = bacc.Bacc("TRN2", target_bir_lowering=False, debug=False)

    xt_d = nc.dram_tensor("xt", [HID, SK], BF16, kind="ExternalInput").ap()
    wqt_d = nc.dram_tensor("wqt", [HID, 512], BF16, kind="ExternalInput").ap()
    # wkt/wvt arrive pre-tiled [128, HT*128] so the DMA is one straight copy
    wkt_d = nc.dram_tensor("wkt", [128, HT * 128], BF16,
                           kind="ExternalInput").ap()
    wvt_d = nc.dram_tensor("wvt", [128, HT * 128], BF16,
                           kind="ExternalInput").ap()
    wot_d = nc.dram_tensor("wot", [512, HID], BF16, kind="ExternalInput").ap()
    csa_d = nc.dram_tensor("csa", [128, SK], BF16, kind="ExternalInput").ap()
    csb_d = nc.dram_tensor("csb", [128, SK], BF16, kind="ExternalInput").ap()
    comb_d = nc.dram_tensor("comb", [128, 256], BF16, kind="ExternalInput").ap()
    ident_d = nc.dram_tensor("ident", [128, 128], BF16, kind="ExternalInput").ap()
    onesm_d = nc.dram_tensor("onesm", [128, SK], BF16, kind="ExternalInput").ap()
    bnd_d = nc.dram_tensor("bnd", [128, 256], BF16, kind="ExternalInput").ap()
    out_d = nc.dram_tensor("out", [HID, SQ], BF16, kind="ExternalOutput").ap()

    with tile.TileContext(nc) as tc:
        with tc.tile_pool(name="persist", bufs=1) as pp:
            x_sb = pp.tile([128, HT * SK], BF16)       # 48KB/part, resident x
            wkt_sb = pp.tile([128, HT * 128], BF16)    # 4KB
            wvt_sb = pp.tile([128, HT * 128], BF16)    # 4KB
            wqt_sb = pp.tile([128, HT * 512], BF16)    # 16KB
            wot_sb = pp.tile([128, NHC * HID], BF16)   # 16KB
            csa_sb = pp.tile([128, SK], BF16)
            csb_sb = pp.tile([128, SK], BF16)
            comb_sb = pp.tile([128, 256], BF16)
            ident_sb = pp.tile([128, 128], BF16)
            onesm_sb = pp.tile([128, SK], BF16)
            bnd_sb = pp.tile([128, 256], BF16)
            kt_rot = pp.tile([128, SK], BF16)
            vt_sb = pp.tile([128, SK], BF16)           # v staging (d-major)
            vs_sb = pp.tile([128, SK], BF16)           # v s-major
            qt_rot = pp.tile([128, NHC * SQ], BF16)    # 8KB
            attnT = pp.tile([128, NHC * SQ], BF16)     # 8KB
            m1_sb = pp.tile([128, SQ], BF16)
            m2_sb = pp.tile([128, SQ], BF16)
            m1b_sb = pp.tile([128, SQ], BF16)
            m2b_sb = pp.tile([128, SQ], BF16)
            recip_sb = pp.tile([128, 512], F32)
            pblocks = [pp.tile([128, P_TOTAL], BF16, name=f"pblock{i}")
                       for i in range(2)]

            # ---- input DMAs (issue order ~= need order) ----
            nc.sync.dma_start(x_sb[:, 0:SK], xt_d[0:128, :])
            nc.sync.dma_start(wkt_sb[:], wkt_d)
            nc.sync.dma_start(wvt_sb[:], wvt_d)
            nc.sync.dma_start(x_sb[:, SK:2 * SK], xt_d[128:256, :])
            nc.sync.dma_start(comb_sb[:], comb_d)
            nc.sync.dma_start(ident_sb[:], ident_d)
            for ht in range(2, HT):
                nc.sync.dma_start(x_sb[:, SK * ht:SK * (ht + 1)],
                                  xt_d[128 * ht:128 * (ht + 1), :])
            nc.sync.dma_start(csa_sb[:], csa_d)
            nc.sync.dma_start(csb_sb[:], csb_d)
            for _t in range(HT):
                nc.sync.dma_start(
                    wqt_sb[:, 512 * _t:512 * (_t + 1)],
                    wqt_d[128 * _t:128 * (_t + 1), :])
            nc.sync.dma_start(onesm_sb[:], onesm_d)
            nc.sync.dma_start(bnd_sb[:], bnd_d)
            for _m in range(NHC):
                nc.sync.dma_start(
                    wot_sb[:, HID * _m:HID * (_m + 1)],
                    wot_d[128 * _m:128 * (_m + 1), :])

            # ================= phase A: k/v projections =================
            with tc.tile_pool(name="kvps", bufs=1, space="PSUM") as kvps, \
                 tc.tile_pool(name="rotps", bufs=2, space="PSUM") as rotps:
                k_ps = kvps.tile([128, SK], F32, tag="k")
                v_ps = kvps.tile([128, SK], F32, tag="v")
                for ht in range(HT):
                    for sc in range(3):
                        sl = slice(512 * sc, 512 * (sc + 1))
                        xsl = x_sb[:, SK * ht + 512 * sc:
                                   SK * ht + 512 * (sc + 1)]
                        nc.tensor.matmul(k_ps[:, sl],
                                         wkt_sb[:, 128 * ht:128 * (ht + 1)],
                                         xsl,
                                         start=(ht == 0), stop=(ht == HT - 1))
                        nc.tensor.matmul(v_ps[:, sl],
                                         wvt_sb[:, 128 * ht:128 * (ht + 1)],
                                         xsl,
                                         start=(ht == 0), stop=(ht == HT - 1))
                # rope(k): DVE muls + constant combine matmul back into psum
                for sc in range(3):
                    sl = slice(512 * sc, 512 * (sc + 1))
                    ma = m1_sb if sc % 2 == 0 else m1b_sb
                    mb = m2_sb if sc % 2 == 0 else m2b_sb
                    nc.vector.tensor_mul(ma[:, 0:512], k_ps[:, sl],
                                         csa_sb[:, sl])
                    nc.vector.tensor_mul(mb[:, 0:512], k_ps[:, sl],
                                         csb_sb[:, sl])
                    nc.tensor.matmul(k_ps[:, sl], comb_sb[:, 0:128],
                                     ma[:, 0:512], start=True, stop=False)
                    nc.tensor.matmul(k_ps[:, sl], comb_sb[:, 128:256],
                                     mb[:, 0:512], start=False, stop=True)
                    nc.scalar.copy(kt_rot[:, sl], k_ps[:, sl])
                # v: psum -> sbuf (d-major), transpose to s-major
                for sc in range(3):
                    sl = slice(512 * sc, 512 * (sc + 1))
                    nc.vector.tensor_copy(vt_sb[:, sl], v_ps[:, sl])
                for tj in range(TJ):
                    sl = slice(128 * tj, 128 * (tj + 1))
                    t_ps = rotps.tile([128, 128], BF16, tag="tr")
                    nc.tensor.transpose(t_ps[:], vt_sb[:, sl], ident_sb[:])
                    nc.vector.tensor_copy(vs_sb[:, sl], t_ps[:])

            # ================ phase B: q projection + rope ================
            with tc.tile_pool(name="qps", bufs=3, space="PSUM") as qps:
                q_ps = [None] * NHC

                def qproj_head(h):
                    q_ps[h] = qps.tile([128, SQ], F32, tag="q",
                                       name=f"q_ps{h}")
                    for ht in range(HT):
                        for sc in range(2):
                            nc.tensor.matmul(
                                q_ps[h][:, 512 * sc:512 * (sc + 1)],
                                wqt_sb[:, 512 * ht + 128 * h:
                                       512 * ht + 128 * (h + 1)],
                                x_sb[:, SK * ht + 512 + 512 * sc:
                                     SK * ht + 512 + 512 * (sc + 1)],
                                start=(ht == 0), stop=(ht == HT - 1))

                def qrope_head(h):
                    ma = m1_sb if h % 2 == 0 else m1b_sb
                    mb = m2_sb if h % 2 == 0 else m2b_sb
                    nc.vector.tensor_mul(ma[:], q_ps[h][:], csa_sb[:, 512:SK])
                    nc.vector.tensor_mul(mb[:], q_ps[h][:], csb_sb[:, 512:SK])
                    for sc in range(2):
                        sl = slice(512 * sc, 512 * (sc + 1))
                        nc.tensor.matmul(q_ps[h][:, sl], comb_sb[:, 0:128],
                                         ma[:, sl], start=True, stop=False)
                        nc.tensor.matmul(q_ps[h][:, sl], comb_sb[:, 128:256],
                                         mb[:, sl], start=False, stop=True)
                    nc.scalar.copy(qt_rot[:, SQ * h:SQ * (h + 1)], q_ps[h][:])

                qproj_head(0)
                qproj_head(1)
                qrope_head(0)
                qproj_head(2)
                qrope_head(1)
                qproj_head(3)
                qrope_head(2)
                qrope_head(3)

            # ================= phase C: banded attention =================
            with tc.tile_pool(name="sps", bufs=2, space="PSUM") as sps, \
                 tc.tile_pool(name="ops", bufs=2, space="PSUM") as ops:
                def scores_part(h):
                    pblock = pblocks[h % 2]
                    # scores + exp + masks; tj=4 and tj=8 first: they are the
                    # start=True full-coverage tiles gating the chunk psums
                    for tj in (4, 8, 0, 1, 2, 3, 5, 6, 7, 9, 10, 11):
                        w0, w1 = _win(tj)
                        W = w1 - w0
                        s_ps = sps.tile([128, 640], F32, tag="s")
                        ktile = kt_rot[:, 128 * tj:128 * (tj + 1)]
                        qv = qt_rot[:, SQ * h + w0:SQ * h + w1]
                        if W <= 512:
                            nc.tensor.matmul(s_ps[:, 0:W], ktile, qv,
                                             start=True, stop=True)
                        else:
                            nc.tensor.matmul(s_ps[:, 0:512], ktile,
                                             qv[:, 0:512],
                                             start=True, stop=True)
                            nc.tensor.matmul(s_ps[:, 512:W], ktile,
                                             qv[:, 512:W],
                                             start=True, stop=True)
                        pt = pblock[:, _P_OFF[tj]:_P_OFF[tj] + W]
                        nc.scalar.activation(pt, s_ps[:, 0:W], EXP)
                        if tj >= 4:
                            nc.gpsimd.tensor_mul(pt[:, 0:128], pt[:, 0:128],
                                                 bnd_sb[:, 0:128])
                        if tj <= 7:
                            nc.gpsimd.tensor_mul(pt[:, W - 128:W],
                                                 pt[:, W - 128:W],
                                                 bnd_sb[:, 128:256])

                def chunks_part(h):
                    pblock = pblocks[h % 2]
                    # denominator + PV accumulation per 512-query chunk
                    for c in range(2):
                        o_ps = ops.tile([128, 512], F32, tag="o")
                        den_ps = ops.tile([128, 512], F32, tag="den")
                        order = [4 * c + 4] + [4 * c + k
                                               for k in (0, 1, 2, 3, 5, 6, 7)]
                        for idx, tj in enumerate(order):
                            w0, w1 = _win(tj)
                            W = w1 - w0
                            lo = max(0, 512 * c - w0)
                            hi = min(W, 512 * c + 512 - w0)
                            pc = slice(w0 + lo - 512 * c, w0 + hi - 512 * c)
                            prhs = pblock[:, _P_OFF[tj] + lo:_P_OFF[tj] + hi]
                            st, sp = idx == 0, idx == len(order) - 1
                            nc.tensor.matmul(den_ps[:, pc],
                                             onesm_sb[:, 128 * tj:
                                                      128 * (tj + 1)],
                                             prhs, start=st, stop=sp,
                                             skip_group_check=True)
                            nc.tensor.matmul(o_ps[:, pc],
                                             vs_sb[:, 128 * tj:128 * (tj + 1)],
                                             prhs, start=st, stop=sp,
                                             skip_group_check=True)
                        nc.vector.reciprocal_approx_fast(recip_sb[:],
                                                         den_ps[:])
                        nc.vector.tensor_mul(
                            attnT[:, SQ * h + 512 * c:SQ * h + 512 * (c + 1)],
                            o_ps[:], recip_sb[:])

                scores_part(0)
                for h in range(1, NHC):
                    scores_part(h)
                    chunks_part(h - 1)
                chunks_part(NHC - 1)

            # ============ phase D: output projection (transposed) ============
            with tc.tile_pool(name="ost", bufs=1) as ostp, \
                 tc.tile_pool(name="fps", bufs=4, space="PSUM") as fps:
                for ot in range(HID // 128):
                    f_ps = fps.tile([128, SQ], F32, tag="f", name=f"f_ps{ot}")
                    for m in range(NHC):
                        wslice = wot_sb[:, HID * m + 128 * ot:
                                        HID * m + 128 * (ot + 1)]
                        for sc in range(2):
                            nc.tensor.matmul(
                                f_ps[:, 512 * sc:512 * (sc + 1)],
                                wslice,
                                attnT[:, SQ * m + 512 * sc:
                                      SQ * m + 512 * (sc + 1)],
                                start=(m == 0), stop=(m == NHC - 1),
                                skip_group_check=True)
                    stage = ostp.tile([128, SQ], BF16, tag="st", bufs=4)
                    for sc in range(2):
                        ssl = slice(512 * sc, 512 * (sc + 1))
                        if (ot + sc) % 2 == 0:
                            nc.scalar.copy(stage[:, ssl], f_ps[:, ssl])
                        else:
                            nc.vector.tensor_copy(stage[:, ssl], f_ps[:, ssl])
                        nc.sync.dma_start(
                            out_d[128 * ot:128 * (ot + 1), ssl],
                            stage[:, ssl])

    nc.compile()
    return nc


def host_inputs(x, wq, wk, wv, wo, freqs_cos, freqs_sin):
    """Build the 8 per-core input dicts (all bf16)."""
    xT = np.ascontiguousarray(np.asarray(x, dtype=np.float32)[0].T)  # [hid, s]
    wq = np.asarray(wq, dtype=np.float32)
    wk = np.asarray(wk, dtype=np.float32)
    wv = np.asarray(wv, dtype=np.float32)
    wo = np.asarray(wo, dtype=np.float32)
    cosT = np.asarray(freqs_cos, dtype=np.float32).T  # [64, S]
    sinT = np.asarray(freqs_sin, dtype=np.float32).T

    comb = np.zeros((128, 256), dtype=np.float32)
    for p in range(64):
        comb[p, p] = 1.0        # C1: out[p] = m1[p] - m1[p+64]
        comb[64 + p, p] = -1.0
        comb[p, 128 + 64 + p] = 1.0   # C2: out[64+p] = m2[p] + m2[p+64]
        comb[64 + p, 128 + 64 + p] = 1.0
    ident = np.eye(128, dtype=np.float32)
    y = np.arange(128)[None, :]
    xg = np.arange(128)[:, None]
    bnd = np.concatenate([(y >= xg).astype(np.float32),
                          (y <= xg).astype(np.float32)], axis=1)  # [128, 256]

    def b16(a):
        return np.ascontiguousarray(a.astype(NPBF16))

    in_maps = []
    for core in range(NCORES):
        g, r = core // 2, core % 2
        lo = 1024 * r - 512
        xt = np.zeros((HID, SK), dtype=np.float32)
        if r == 0:
            xt[:, 512:] = xT[:, 0:1024]
        else:
            xt[:, :] = xT[:, 512:2048]
        pos = np.clip(np.arange(lo, lo + SK), 0, S - 1)
        csa = np.concatenate([cosT[:, pos], sinT[:, pos]], axis=0)
        csb = np.concatenate([sinT[:, pos], cosT[:, pos]], axis=0)
        onesm = np.zeros((128, SK), dtype=np.float32)
        for tj in range(TJ):
            real = np.ones(128, dtype=np.float32) if r == 1 else \
                (128 * tj + np.arange(128) >= 512).astype(np.float32)
            onesm[:, 128 * tj:128 * (tj + 1)] = real[:, None]
        wktT = wk[128 * g:128 * (g + 1), :].T / np.sqrt(HD)   # [2048, 128]
        wvtT = wv[128 * g:128 * (g + 1), :].T
        # pre-tile [2048, 128] -> [128, 16*128]: partition p, tile t holds
        # row 128t+p
        wkt_tiled = wktT.reshape(HT, 128, 128).transpose(1, 0, 2).reshape(
            128, HT * 128)
        wvt_tiled = wvtT.reshape(HT, 128, 128).transpose(1, 0, 2).reshape(
            128, HT * 128)
        in_maps.append({
            "xt": b16(xt),
            "wqt": b16(wq[512 * g:512 * (g + 1), :].T),
            "wkt": b16(wkt_tiled),
            "wvt": b16(wvt_tiled),
            "wot": b16(wo[:, 512 * g:512 * (g + 1)].T),
            "csa": b16(csa),
            "csb": b16(csb),
            "comb": b16(comb),
            "ident": b16(ident),
            "onesm": b16(onesm),
            "bnd": b16(bnd),
        })
    return in_maps


def reduce_outputs(results):
    out = np.zeros((S, HID), dtype=np.float32)
    for core, res in enumerate(results):
        r = core % 2
        out[1024 * r:1024 * (r + 1), :] += \
            np.asarray(res["out"], dtype=np.float32).T
    return out[None]


_NC = None
_IN_MAPS = None


def _numpy_fallback(x, wq, wk, wv, wo, attention_mask, freqs_cos, freqs_sin):
    """Exact (slow) path for non-causal attention_mask inputs."""
    xs = np.asarray(x, np.float32)[0]
    cos = np.asarray(freqs_cos, np.float32)
    sin = np.asarray(freqs_sin, np.float32)

    def rope(t):
        x1, x2 = t[..., :64], t[..., 64:]
        c, s = cos[:, None, :], sin[:, None, :]
        return np.concatenate([x1 * c - x2 * s, x1 * s + x2 * c], axis=-1)

    q = rope((xs @ np.asarray(wq, np.float32).T).reshape(S, NH, HD))
    k = rope((xs @ np.asarray(wk, np.float32).T).reshape(S, NKV, HD))
    v = (xs @ np.asarray(wv, np.float32).T).reshape(S, NKV, HD)
    k = np.repeat(k, NH // NKV, axis=1)
    v = np.repeat(v, NH // NKV, axis=1)
    i = np.arange(S)[:, None]
    j = np.arange(S)[None, :]
    wmask = (i - j) > WINDOW
    out = np.zeros((S, NH, HD), np.float32)
    am = np.asarray(attention_mask, np.float32)[0, 0]
    for h in range(NH):
        sc = (q[:, h] @ k[:, h].T) / np.sqrt(HD) + am
        sc = np.where(wmask, -np.inf, sc)
        sc -= sc.max(axis=1, keepdims=True)
        p = np.exp(sc)
        p /= p.sum(axis=1, keepdims=True)
        out[:, h] = p @ v[:, h]
    return (out.reshape(S, NH * HD) @ np.asarray(wo, np.float32).T)[None]


def _is_standard_causal(attention_mask):
    am = np.asarray(attention_mask)
    if am.shape != (1, 1, S, S):
        return False
    i = np.arange(S)[:, None]
    j = np.arange(S)[None, :]
    expect = np.where(j > i, np.float32(-1e9), np.float32(0.0))
    return np.array_equal(am[0, 0], expect)


def kernel(x, wq, wk, wv, wo, attention_mask, freqs_cos, freqs_sin,
           **extra):
    global _NC, _IN_MAPS
    if not _is_standard_causal(attention_mask):
        return _numpy_fallback(x, wq, wk, wv, wo, attention_mask,
                               freqs_cos, freqs_sin)
    in_maps = host_inputs(x, wq, wk, wv, wo, freqs_cos, freqs_sin)
    _IN_MAPS = in_maps
    if _NC is None:
        _NC = build_nc()
    res = run_bass_kernel_spmd(_NC, in_maps, core_ids=list(range(NCORES)))
    return reduce_outputs(res.results)


if __name__ == "__main__":
    nc = build_nc()
    print("kernel built OK")
